# revision 3
# baseline (speedup 1.0000x reference)
"""BatchAllTripletLoss on 8 Trainium2 NeuronCores — v3.

Per-iteration critical path (For_i has an all-engine barrier, so the
loop bench == repeated single-shot):
  input DMA (3 parallel queues; small lm2/rhsx/mb queue lands first)
  -> PE g_w (early inputs) -> ACT band/dband -> DVE pos     [hidden]
  -> PE g_n (big inputs)   -> ACT dn = sqrt -> DVE ndn = -dn
  -> hot loop: n_dve slots on DVE (ONE fused custom-DVE instr each:
     acc = sum relu(ndn+pos_j) + 8192*count; 1x but does both) and
     n_act slots on ACT (Relu+Sign pair, scale=-1 on dn).
  -> out DMA.

Final config (measured For_i-differential, DI=20000): fp8 e4m3 grams
(extras chunk stays f16; rel err 1.0e-3 vs 2e-2 gate), lhsT dropped
(w-gram = lm2^T lm2, rescaled -0.5 in the band ACT), lm2+mb+rhsa
packed into ONE sync-queue DMA (bitcast views), rhsb+rhsx on the ACT
queue, ONE n-side PSUM bank + ONE sqrt -> f16 dn, dn-direct custom op
relu(C0 - Src0), hot-slot balance 17 DVE / 7 ACT.
"""
import sys
sys.path.insert(0, "/opt/trn_rl_repo")

import numpy as np
from contextlib import ExitStack

import concourse.bass as bass
import concourse.tile as tile
from concourse import bacc, mybir
from concourse.bass_utils import run_bass_kernel_spmd

F32 = mybir.dt.float32
F16 = mybir.dt.float16
Alu = mybir.AluOpType
Act = mybir.ActivationFunctionType

B = 512
K = 4            # contraction chunks of 128 (512 dims)
NB = 512         # negative columns per core
NH = 256
N_CORES = 8
OH = 1000.0      # onehot amplitude -> 1e6 mask in the gram
NEG = -3032.0    # pos offset for invalid (row, slot) pairs

_cache = {}

CNT_SCALE = 8192.0     # fused accum: acc = sum relu(x) + CNT_SCALE*count


def _register_fused_op():
    """Runtime-register a custom DVE op fusing relu-sum + count:
    acc = sum_n[ relu(in0+s0) + CNT_SCALE*(in0+s0 > s1) ]. Decode on host:
    count = floor(acc/CNT_SCALE) (valid since sum relu < CNT_SCALE)."""
    import concourse.dve_ops as dve_ops
    from concourse.dve_spec import Spec, Src0, C0, C1, C2, relu, lower, \
        _has_src1
    from concourse.dve_ops import DveOp
    from concourse.dve_uop import DveOpSpec
    name = "RELU_RSUB_CNT_ANT"
    if name in dve_ops._SUB_OPCODE_FOR_NAME:
        for o in dve_ops.OPS:
            if o.name == name:
                return o
    spec = Spec(
        body=relu(C0 - Src0) + ((C0 - Src0) > C1) * C2,
        accum=__import__("operator").add,
        reference=lambda in0, s0, s1, imm2: (
            np.maximum(s0 - in0.astype(np.float32), 0.0)
            + imm2 * ((s0 - in0.astype(np.float32)) > s1)),
    )
    op = DveOp(name, spec, subdim=False, uops_sha={})
    row = dve_ops._CUSTOM_DVE_ROW_BASE + len(dve_ops.OPS)
    assert row < 0x20
    dve_ops.OPS.append(op)
    dve_ops.CUSTOM_DVE_SPECS[name] = spec
    dve_ops._SUB_OPCODE_FOR_NAME[name] = row
    for ver in ("v3", "v4"):
        s = DveOpSpec(name=name, opcode=row, uops=lower(spec, ver=ver),
                      rd1_en=_has_src1(spec))
        object.__setattr__(op, "uops_sha", {**op.uops_sha, ver: s.sha(ver)})
    return op


def _build(S: int, R: int, C: int, n_act: int = 7, n_warm: int = 7,
           dma_gp: bool = False, split_gn: bool = False, fp8: bool = True,
           pack: bool = True, zd_bufs: int = 3, za_bufs: int = 2,
           pooloff: bool = False,
           abl: frozenset = frozenset(),
           loop_iters: int | None = None):
    """S = band width (max class size), R = rows per core (32-aligned
    class slots), C = number of classes."""
    SLOT = 32 * ((S + 31) // 32)
    nslot = R // SLOT
    n_dve = S - n_act
    assert C <= 32 and R % SLOT == 0 and n_dve >= 0

    nc = bacc.Bacc("TRN2", target_bir_lowering=False, debug=False,
                   num_devices=N_CORES)

    FE = mybir.dt.float8e4 if fp8 else F16
    esz = 1 if fp8 else 2
    U8 = mybir.dt.uint8
    if pack:
        # [lm2 | mb | rhsa] packed bytes, one DMA on the sync queue
        PW = K * R * esz + 2 * S + K * NH * esz
        packA_d = nc.dram_tensor("packA", [128, PW], U8,
                                 kind="ExternalInput")
    else:
        lm2_d = nc.dram_tensor("lm2", [128, K * R], FE,
                               kind="ExternalInput")
        rhsa_d = nc.dram_tensor("rhsa", [128, K * NH], FE,
                                kind="ExternalInput")
        mb_d0 = None
    rhsb_d = nc.dram_tensor("rhsb", [128, K * NH], FE,
                            kind="ExternalInput")
    rhsx_d = nc.dram_tensor("rhsx", [33, NB + 2 * R], F16,
                            kind="ExternalInput")
    if not pack:
        mb_d = nc.dram_tensor("mb", [R, S], F16, kind="ExternalInput")
    out_d = nc.dram_tensor("out", [R, S + n_act], F32,
                           kind="ExternalOutput")

    with tile.TileContext(nc) as tc, ExitStack() as ctx:
        pool = ctx.enter_context(tc.tile_pool(name="sbuf", bufs=2))
        spool = ctx.enter_context(tc.tile_pool(name="scr", bufs=3))
        ppool = ctx.enter_context(tc.tile_pool(name="psum", bufs=2, space="PSUM"))

        def _body():
            # ---- input DMAs: 3 queues; small queue (gpsimd) lands first
            rhsb_t = pool.tile([128, K, NH], FE)
            rhsx_t = pool.tile([33, NB + 2 * R], F16)
            if pack:
                PW = K * R * esz + 2 * S + K * NH * esz
                packA_t = pool.tile([128, PW], U8)
                nc.sync.dma_start(packA_t[:], packA_d.ap())
                o1 = K * R * esz
                o2 = o1 + 2 * S
                lm2_t = packA_t[:, 0:o1].bitcast(FE)
                mb_t = packA_t[0:R, o1:o2].bitcast(F16)
                rhsa_flat = packA_t[:, o2:PW].bitcast(FE)
                nc.scalar.dma_start(rhsx_t[:], rhsx_d.ap())
                nc.scalar.dma_start(rhsb_t[:], rhsb_d.ap())
            else:
                lm2_t = pool.tile([128, K * R], FE)
                rhsa_t = pool.tile([128, K, NH], FE)
                mb_t = pool.tile([R, S], F16)
                nc.sync.dma_start(lm2_t[:], lm2_d.ap())
                nc.scalar.dma_start(rhsx_t[:], rhsx_d.ap())
                nc.sync.dma_start(mb_t[:], mb_d.ap())
                nc.sync.dma_start(rhsa_t[:], rhsa_d.ap())
                nc.scalar.dma_start(rhsb_t[:], rhsb_d.ap())

            lm2 = [lm2_t[:, k * R:(k + 1) * R] for k in range(K)]
            mb_ap = mb_t if pack else mb_t[:]
            if pack:
                rhs = {0: [rhsa_flat[:, k * NH:(k + 1) * NH]
                           for k in range(K)],
                       1: [rhsb_t[:, k, :] for k in range(K)]}
            else:
                rhs = {0: [rhsa_t[:, k, :] for k in range(K)],
                       1: [rhsb_t[:, k, :] for k in range(K)]}
            lhs5 = rhsx_t[0:33, NB:NB + R]          # onehot + ones row
            lhs5_ones = rhsx_t[32:33, NB:NB + R]    # ones row
            lhs6_nrm = rhsx_t[32:33, NB + R:NB + 2 * R]  # -2|a|^2 row
            rhs5 = {0: rhsx_t[0:33, 0:NH], 1: rhsx_t[0:33, NH:NB]}

            if 'empty' in abl:
                out_e = pool.tile([R, n_dve + 2 * n_act], F32)
                nc.vector.memset(out_e[:], 1.0)
                nc.sync.dma_start(out_d.ap(), out_e[:])
                return
            # ---- PE warmup during DMA: ramp out of low p-state ----
            one16 = pool.tile([33, 1], F16)
            nc.vector.memset(one16[:], 1.0)
            dumt = pool.tile([1, 2], F32)
            nc.vector.memset(dumt[:], 1.0)
            # pin the ACT table set (sqrt_and_others has sqrt/relu/sign)
            dums = pool.tile([1, 2], F32)
            nc.scalar.activation(dums[:], dumt[:], Act.Sqrt)
            if n_warm:
                wsrc = pool.tile([128, NB], F16)
                nc.gpsimd.memset(wsrc[:], 0.0)
                warm = ppool.tile([1, NB], F32, tag="warm", name="warm",
                                  bufs=1)
                for w in range(n_warm):
                    nc.tensor.matmul(warm[:], wsrc[:, 0:1], wsrc[:],
                                     start=True, stop=True)

            # ---- anchor-norm column: -2|a|^2 row x ones -> x(-0.5) ----
            nrma = ppool.tile([R, 1], F32, tag="nrma", name="nrma", bufs=1)
            nc.tensor.matmul(nrma[:], lhs6_nrm, one16[32:33, :],
                             start=True, stop=True)
            nrma_s = pool.tile([R, 1], F32)
            nc.vector.tensor_scalar(out=nrma_s[:], in0=nrma[:],
                                    scalar1=-0.5, scalar2=None,
                                    op0=Alu.mult)

            # ---- w-side gram (early inputs only): 4 e^T e - 2|a_c|^2 ----
            g_w = ppool.tile([R, R], F32, tag="gw", name="gw")
            for k in range(K):
                nc.tensor.matmul(g_w[:], lm2[k], lm2[k],
                                 start=(k == 0), stop=False)
            nc.tensor.matmul(g_w[:], lhs5_ones, lhs6_nrm,
                             start=False, stop=True)
            # band rects: Relu(-0.5*g_w + |a|^2) = d^2(a, p)
            band2 = pool.tile([R, S], F32)
            for k in range(nslot):
                nc.scalar.activation(
                    band2[k * SLOT:(k + 1) * SLOT, :],
                    g_w[k * SLOT:(k + 1) * SLOT, k * SLOT:k * SLOT + S],
                    Act.Relu, bias=nrma_s[k * SLOT:(k + 1) * SLOT, :],
                    scale=-0.5)
            dband = pool.tile([R, S], F32)
            nc.scalar.activation(dband[:], band2[:], Act.Sqrt)
            pos = pool.tile([R, S], F32)
            if pooloff:
                nc.gpsimd.tensor_tensor(out=pos[:], in0=dband[:], in1=mb_ap,
                                        op=Alu.add)
            else:
                nc.vector.tensor_tensor(out=pos[:], in0=dband[:], in1=mb_ap,
                                        op=Alu.add)

            # ---- n-side gram; dn = sqrt -> f16 ----
            dn = pool.tile([R, NB], F16)
            if split_gn:
                for h in (0, 1):
                    gh_t = ppool.tile([R, NH], F32, tag=f"gn{h}",
                                      name=f"gn{h}")
                    for k in range(K):
                        nc.tensor.matmul(gh_t[:], lm2[k], rhs[h][k],
                                         start=(k == 0), stop=False)
                    nc.tensor.matmul(gh_t[:], lhs5, rhs5[h],
                                     start=False, stop=True)
                    nc.scalar.activation(dn[:, h * NH:(h + 1) * NH],
                                         gh_t[:], Act.Sqrt,
                                         bias=nrma_s[:], scale=1.0)
            else:
                g_n = ppool.tile([R, NB], F32, tag="gn", name="gn")
                for h in (0, 1):
                    gh = g_n[:, h * NH:(h + 1) * NH]
                    for k in range(K):
                        nc.tensor.matmul(gh, lm2[k], rhs[h][k],
                                         start=(k == 0), stop=False)
                    nc.tensor.matmul(gh, lhs5, rhs5[h], start=False,
                                     stop=True)
                nc.scalar.activation(dn[:], g_n[:], Act.Sqrt,
                                     bias=nrma_s[:], scale=1.0)

            # ---- hot loop (separate accum tiles per engine) ----
            fop = _register_fused_op()
            out_t = pool.tile([R, n_dve + 2 * n_act], F32)
            acc_a = pool.tile([R, 2 * max(n_act, 1)], F32)
            if 'no_hot' in abl:
                nc.vector.memset(out_t[:], 1.0)
            for j in (range(0) if 'no_hot' in abl else range(S)):
                pj = pos[:, j:j + 1]
                if j < n_dve:        # custom DVE: fused relu-sum + count
                    z = spool.tile([R, NB], F16, tag="zd", name=f"zd_{j}",
                                   bufs=zd_bufs)
                    nc.vector._custom_dve(
                        fop, out=z[:], in0=dn[:], s0=pj, s1=0.0,
                        imm2=CNT_SCALE, accum_out=out_t[:, j:j + 1])
                else:                # ACT: Relu sum + Sign count on dn
                    ja = j - n_dve
                    z1 = spool.tile([R, NB], F16, tag="za", name=f"za1_{j}",
                                    bufs=za_bufs)
                    nc.scalar.activation(z1[:], dn[:], Act.Relu,
                                         bias=pj, scale=-1.0,
                                         accum_out=acc_a[:, 2 * ja:2 * ja + 1])
                    z2 = spool.tile([R, NB], F16, tag="za", name=f"za2_{j}",
                                    bufs=za_bufs)
                    nc.scalar.activation(z2[:], dn[:], Act.Sign,
                                         bias=pj, scale=-1.0,
                                         accum_out=acc_a[:, 2 * ja + 1:2 * ja + 2])
            if n_act and 'no_hot' not in abl:
                (nc.gpsimd if pooloff else nc.vector).tensor_copy(
                    out_t[:, n_dve:n_dve + 2 * n_act], acc_a[:])

            nc.sync.dma_start(out_d.ap(), out_t[:])

        if loop_iters is None:
            _body()
        else:
            with tc.For_i(0, loop_iters, 1):
                _body()

    nc.compile()
    return nc


def _ilv(a, nchunk):
    """[nchunk*128 (contraction), x] -> [128, nchunk*x] chunk-interleaved."""
    x = a.shape[1]
    return np.ascontiguousarray(
        a.reshape(nchunk, 128, x).transpose(1, 0, 2).reshape(128, nchunk * x))


def _prepare(embeddings: np.ndarray, labels: np.ndarray,
             fp8: bool = True, pack: bool = True):
    emb = np.ascontiguousarray(np.asarray(embeddings, dtype=np.float32))
    lab = np.asarray(labels)

    perm = np.argsort(lab, kind="stable")
    e_s = emb[perm]
    lab_s = lab[perm]
    classes, starts, counts = np.unique(lab_s, return_index=True,
                                        return_counts=True)
    C = len(classes)
    S = int(counts.max())
    SLOT = 32 * ((S + 31) // 32)
    spb = -(-C // N_CORES)            # class slots per block
    R = spb * SLOT
    assert R <= 128, f"padded rows per core {R} > 128"

    cls_of_col = np.searchsorted(starts, np.arange(B), side="right") - 1

    # padded anchors: class c -> slot c, rows [SLOT*c, SLOT*c+m_c)
    nP = N_CORES * R
    eP = np.zeros((nP, B), dtype=np.float32)
    cls_of_row = np.repeat(np.arange(-(-nP // SLOT)), SLOT)[:nP]
    live_row = np.zeros((nP,), dtype=bool)
    for c in range(C):
        eP[SLOT * c:SLOT * c + counts[c]] = \
            e_s[starts[c]:starts[c] + counts[c]]
        live_row[SLOT * c:SLOT * c + counts[c]] = True
    ePT = np.ascontiguousarray(eP.T).astype(np.float16)
    e_sT = np.ascontiguousarray(e_s.T).astype(np.float16)  # [512, 512]

    sqa_all = (ePT.astype(np.float32) ** 2).sum(0)         # [nP]
    sqn = (e_sT.astype(np.float32) ** 2).sum(0)            # [512]

    # rhs extra chunk: onehot rows + |b|^2 in row 32
    ch_r = np.zeros((33, NB), dtype=np.float16)
    ch_r[cls_of_col[:NB], np.arange(NB)] = OH
    ch_r[32, :] = sqn.astype(np.float16)

    rhsa = _ilv(e_sT[:, :NH], K)
    rhsb = _ilv(e_sT[:, NH:], K)

    num_valid = float((counts * (counts - 1) * (B - counts)).sum())

    in_maps = []
    for b in range(N_CORES):
        cols = np.arange(R * b, R * b + R)
        livec = live_row[cols]
        row_cls = np.minimum(cls_of_row[cols], C - 1)
        row_m = counts[row_cls]
        # lhs extra chunk: onehot rows (anchor class), ones row
        ch = np.zeros((33, R), dtype=np.float16)
        ch[row_cls, np.arange(R)] = np.where(livec, OH, 0.0)
        ch[32, :] = np.float16(1.0)
        # second extras col-block: row 32 = -2|a|^2
        ch6 = np.zeros((33, R), dtype=np.float16)
        ch6[32, :] = (-2.0 * sqa_all[cols]).astype(np.float16)
        lhs_chunks = ePT[:, cols].reshape(K, 128, R)
        lhsT = np.concatenate([lhs_chunks[k] for k in range(K)], axis=1)
        lm2 = np.ascontiguousarray((-2.0 * lhsT.astype(np.float32))
                                   .astype(np.float16))
        rhsx = np.ascontiguousarray(
            np.concatenate([ch_r, ch, ch6], axis=1))

        ii = np.tile(np.arange(SLOT), spb)
        jj = np.arange(S)[None, :]
        valid = ((jj < row_m[:, None]) & (jj != ii[:, None])
                 & (ii[:, None] < row_m[:, None]))
        mb = np.where(valid, 0.0, NEG).astype(np.float16)
        if fp8:
            import ml_dtypes
            f8 = ml_dtypes.float8_e4m3fn
            lm2 = lm2.astype(np.float32).astype(f8)
            rhsa_c = rhsa.astype(np.float32).astype(f8)
            rhsb_c = rhsb.astype(np.float32).astype(f8)
        else:
            rhsa_c, rhsb_c = rhsa, rhsb
        if pack:
            packA = np.concatenate(
                [np.ascontiguousarray(lm2).view(np.uint8),
                 np.ascontiguousarray(mb).view(np.uint8),
                 np.ascontiguousarray(rhsa_c).view(np.uint8)], axis=1)
            in_maps.append({
                "packA": np.ascontiguousarray(packA),
                "rhsb": rhsb_c,
                "rhsx": rhsx,
            })
        else:
            in_maps.append({
                "lm2": lm2,
                "rhsa": rhsa_c,
                "rhsb": rhsb_c,
                "rhsx": rhsx,
                "mb": mb,
            })
    return S, R, C, in_maps, num_valid


def _combine(outs, num_valid, S, n_act=7):
    n_dve = S - n_act
    loss_sum = 0.0
    num_pos = 0.0
    R = outs[0].shape[0]
    for c in range(N_CORES):
        o = np.asarray(outs[c], dtype=np.float64)
        # fused DVE columns: count = floor(acc/CNT_SCALE), relu = rest
        fused = o[:, 0:n_dve]
        fcnt = np.floor(fused / CNT_SCALE)
        loss_sum += (fused - CNT_SCALE * fcnt).sum()
        num_pos += fcnt.sum()
        # ACT column pairs: +relu sum, sign count
        for ja in range(n_act):
            c0 = n_dve + 2 * ja
            loss_sum += o[:, c0].sum()
            num_pos += 0.5 * (o[:, c0 + 1].sum() + NB * R)
    loss = np.float32(loss_sum / (num_pos + 1e-5))
    frac = np.float32(num_pos / (num_valid + 1e-5))
    return (loss, frac)


def kernel(embeddings: np.ndarray, labels: np.ndarray):
    S, R, C, in_maps, num_valid = _prepare(embeddings, labels)
    n_act = min(7, max(S - 1, 0))
    key = (S, R, C, n_act)
    if key not in _cache:
        _cache[key] = _build(S, R, C, n_act=n_act)
    nc = _cache[key]
    res = run_bass_kernel_spmd(nc, in_maps, core_ids=list(range(N_CORES)))
    return _combine([res.results[c]["out"] for c in range(N_CORES)],
                    num_valid, S, n_act=n_act)


# revision 4
# speedup vs baseline: 1.0055x; 1.0055x over previous
"""BatchAllTripletLoss on 8 Trainium2 NeuronCores — v3.

Per-iteration critical path (For_i has an all-engine barrier, so the
loop bench == repeated single-shot):
  input DMA (3 parallel queues; small lm2/rhsx/mb queue lands first)
  -> PE g_w (early inputs) -> ACT band/dband -> DVE pos     [hidden]
  -> PE g_n (big inputs)   -> ACT dn = sqrt -> DVE ndn = -dn
  -> hot loop: n_dve slots on DVE (ONE fused custom-DVE instr each:
     acc = sum relu(ndn+pos_j) + 8192*count; 1x but does both) and
     n_act slots on ACT (Relu+Sign pair, scale=-1 on dn).
  -> out DMA.

Final config (measured For_i-differential, DI=20000): fp8 e4m3 grams
(extras chunk stays f16; rel err 1.0e-3 vs 2e-2 gate), lhsT dropped
(w-gram = lm2^T lm2, rescaled -0.5 in the band ACT), lm2+mb+rhsa
packed into ONE sync-queue DMA (bitcast views), rhsb+rhsx on the ACT
queue, ONE n-side PSUM bank + ONE sqrt -> f16 dn, dn-direct custom op
relu(C0 - Src0), hot-slot balance 17 DVE / 7 ACT.
"""
import sys
sys.path.insert(0, "/opt/trn_rl_repo")

import numpy as np
from contextlib import ExitStack

import concourse.bass as bass
import concourse.tile as tile
from concourse import bacc, mybir
from concourse.bass_utils import run_bass_kernel_spmd

F32 = mybir.dt.float32
F16 = mybir.dt.float16
Alu = mybir.AluOpType
Act = mybir.ActivationFunctionType

B = 512
K = 4            # contraction chunks of 128 (512 dims)
NB = 512         # negative columns per core
NH = 256
N_CORES = 8
OH = 1000.0      # onehot amplitude -> 1e6 mask in the gram
NEG = -3032.0    # pos offset for invalid (row, slot) pairs

_cache = {}

CNT_SCALE = 8192.0     # fused accum: acc = sum relu(x) + CNT_SCALE*count


def _register_fused_op():
    """Runtime-register a custom DVE op fusing relu-sum + count:
    acc = sum_n[ relu(in0+s0) + CNT_SCALE*(in0+s0 > s1) ]. Decode on host:
    count = floor(acc/CNT_SCALE) (valid since sum relu < CNT_SCALE)."""
    import concourse.dve_ops as dve_ops
    from concourse.dve_spec import Spec, Src0, C0, C1, C2, relu, lower, \
        _has_src1
    from concourse.dve_ops import DveOp
    from concourse.dve_uop import DveOpSpec
    name = "RELU_RSUB_CNT_ANT"
    if name in dve_ops._SUB_OPCODE_FOR_NAME:
        for o in dve_ops.OPS:
            if o.name == name:
                return o
    spec = Spec(
        body=relu(C0 - Src0) + ((C0 - Src0) > C1) * C2,
        accum=__import__("operator").add,
        reference=lambda in0, s0, s1, imm2: (
            np.maximum(s0 - in0.astype(np.float32), 0.0)
            + imm2 * ((s0 - in0.astype(np.float32)) > s1)),
    )
    op = DveOp(name, spec, subdim=False, uops_sha={})
    row = dve_ops._CUSTOM_DVE_ROW_BASE + len(dve_ops.OPS)
    assert row < 0x20
    dve_ops.OPS.append(op)
    dve_ops.CUSTOM_DVE_SPECS[name] = spec
    dve_ops._SUB_OPCODE_FOR_NAME[name] = row
    for ver in ("v3", "v4"):
        s = DveOpSpec(name=name, opcode=row, uops=lower(spec, ver=ver),
                      rd1_en=_has_src1(spec))
        object.__setattr__(op, "uops_sha", {**op.uops_sha, ver: s.sha(ver)})
    return op


def _build(S: int, R: int, C: int, n_act: int = 7, n_warm: int = 7,
           dma_gp: bool = False, split_gn: bool = False, fp8: bool = True,
           pack: bool = True, zd_bufs: int = 3, za_bufs: int = 2,
           pooloff: bool = False,
           abl: frozenset = frozenset(),
           loop_iters: int | None = None):
    """S = band width (max class size), R = rows per core (32-aligned
    class slots), C = number of classes."""
    SLOT = 32 * ((S + 31) // 32)
    nslot = R // SLOT
    n_dve = S - n_act
    assert C <= 32 and R % SLOT == 0 and n_dve >= 0

    nc = bacc.Bacc("TRN2", target_bir_lowering=False, debug=False,
                   num_devices=N_CORES)

    FE = mybir.dt.float8e4 if fp8 else F16
    esz = 1 if fp8 else 2
    U8 = mybir.dt.uint8
    if pack:
        # [lm2 | mb | rhsa] packed bytes, one DMA on the sync queue
        PW = K * R * esz + 2 * S + K * NH * esz
        packA_d = nc.dram_tensor("packA", [128, PW], U8,
                                 kind="ExternalInput")
    else:
        lm2_d = nc.dram_tensor("lm2", [128, K * R], FE,
                               kind="ExternalInput")
        rhsa_d = nc.dram_tensor("rhsa", [128, K * NH], FE,
                                kind="ExternalInput")
        mb_d0 = None
    rhsb_d = nc.dram_tensor("rhsb", [128, K * NH], FE,
                            kind="ExternalInput")
    rhsx_d = nc.dram_tensor("rhsx", [33, NB + 2 * R], F16,
                            kind="ExternalInput")
    if not pack:
        mb_d = nc.dram_tensor("mb", [R, S], F16, kind="ExternalInput")
    out_d = nc.dram_tensor("out", [R, S + n_act], F32,
                           kind="ExternalOutput")

    with tile.TileContext(nc) as tc, ExitStack() as ctx:
        pool = ctx.enter_context(tc.tile_pool(name="sbuf", bufs=2))
        spool = ctx.enter_context(tc.tile_pool(name="scr", bufs=3))
        ppool = ctx.enter_context(tc.tile_pool(name="psum", bufs=2, space="PSUM"))

        def _body():
            # ---- input DMAs: 3 queues; small queue (gpsimd) lands first
            rhsb_t = pool.tile([128, K, NH], FE)
            rhsx_t = pool.tile([33, NB + 2 * R], F16)
            if pack:
                PW = K * R * esz + 2 * S + K * NH * esz
                packA_t = pool.tile([128, PW], U8)
                nc.sync.dma_start(packA_t[:], packA_d.ap())
                o1 = K * R * esz
                o2 = o1 + 2 * S
                lm2_t = packA_t[:, 0:o1].bitcast(FE)
                mb_t = packA_t[0:R, o1:o2].bitcast(F16)
                rhsa_flat = packA_t[:, o2:PW].bitcast(FE)
                nc.scalar.dma_start(rhsx_t[:], rhsx_d.ap())
                nc.scalar.dma_start(rhsb_t[:], rhsb_d.ap())
            else:
                lm2_t = pool.tile([128, K * R], FE)
                rhsa_t = pool.tile([128, K, NH], FE)
                mb_t = pool.tile([R, S], F16)
                nc.sync.dma_start(lm2_t[:], lm2_d.ap())
                nc.scalar.dma_start(rhsx_t[:], rhsx_d.ap())
                nc.sync.dma_start(mb_t[:], mb_d.ap())
                nc.sync.dma_start(rhsa_t[:], rhsa_d.ap())
                nc.scalar.dma_start(rhsb_t[:], rhsb_d.ap())

            lm2 = [lm2_t[:, k * R:(k + 1) * R] for k in range(K)]
            mb_ap = mb_t if pack else mb_t[:]
            if pack:
                rhs = {0: [rhsa_flat[:, k * NH:(k + 1) * NH]
                           for k in range(K)],
                       1: [rhsb_t[:, k, :] for k in range(K)]}
            else:
                rhs = {0: [rhsa_t[:, k, :] for k in range(K)],
                       1: [rhsb_t[:, k, :] for k in range(K)]}
            lhs5 = rhsx_t[0:33, NB:NB + R]          # onehot + ones row
            lhs5_ones = rhsx_t[32:33, NB:NB + R]    # ones row
            lhs6_nrm = rhsx_t[32:33, NB + R:NB + 2 * R]  # -2|a|^2 row
            rhs5 = {0: rhsx_t[0:33, 0:NH], 1: rhsx_t[0:33, NH:NB]}

            if 'empty' in abl:
                out_e = pool.tile([R, n_dve + 2 * n_act], F32)
                nc.vector.memset(out_e[:], 1.0)
                nc.sync.dma_start(out_d.ap(), out_e[:])
                return
            # ---- PE warmup during DMA: ramp out of low p-state ----
            one16 = pool.tile([33, 1], F16)
            nc.vector.memset(one16[:], 1.0)
            dumt = pool.tile([1, 2], F32)
            nc.vector.memset(dumt[:], 1.0)
            # pin the ACT table set (sqrt_and_others has sqrt/relu/sign)
            dums = pool.tile([1, 2], F32)
            nc.scalar.activation(dums[:], dumt[:], Act.Sqrt)
            if n_warm:
                wsrc = pool.tile([128, NB], F16)
                nc.gpsimd.memset(wsrc[:], 0.0)
                warm = ppool.tile([1, NB], F32, tag="warm", name="warm",
                                  bufs=1)
                for w in range(n_warm):
                    nc.tensor.matmul(warm[:], wsrc[:, 0:1], wsrc[:],
                                     start=True, stop=True)

            # ---- anchor-norm column: -2|a|^2 row x ones -> x(-0.5) ----
            nrma = ppool.tile([R, 1], F32, tag="nrma", name="nrma", bufs=1)
            nc.tensor.matmul(nrma[:], lhs6_nrm, one16[32:33, :],
                             start=True, stop=True)
            nrma_s = pool.tile([R, 1], F32)
            nc.vector.tensor_scalar(out=nrma_s[:], in0=nrma[:],
                                    scalar1=-0.5, scalar2=None,
                                    op0=Alu.mult)

            # ---- w-side gram (early inputs only): 4 e^T e - 2|a_c|^2 ----
            g_w = ppool.tile([R, R], F32, tag="gw", name="gw")
            for k in range(K):
                nc.tensor.matmul(g_w[:], lm2[k], lm2[k],
                                 start=(k == 0), stop=False)
            nc.tensor.matmul(g_w[:], lhs5_ones, lhs6_nrm,
                             start=False, stop=True)
            # band rects: Relu(-0.5*g_w + |a|^2) = d^2(a, p)
            band2 = pool.tile([R, S], F32)
            for k in range(nslot):
                nc.scalar.activation(
                    band2[k * SLOT:(k + 1) * SLOT, :],
                    g_w[k * SLOT:(k + 1) * SLOT, k * SLOT:k * SLOT + S],
                    Act.Relu, bias=nrma_s[k * SLOT:(k + 1) * SLOT, :],
                    scale=-0.5)
            dband = pool.tile([R, S], F32)
            nc.scalar.activation(dband[:], band2[:], Act.Sqrt)
            pos = pool.tile([R, S], F32)
            if pooloff:
                nc.gpsimd.tensor_tensor(out=pos[:], in0=dband[:], in1=mb_ap,
                                        op=Alu.add)
            else:
                nc.vector.tensor_tensor(out=pos[:], in0=dband[:], in1=mb_ap,
                                        op=Alu.add)

            # ---- n-side gram; dn = sqrt -> f16 ----
            dn = pool.tile([R, NB], F16)
            if split_gn:
                for h in (0, 1):
                    gh_t = ppool.tile([R, NH], F32, tag=f"gn{h}",
                                      name=f"gn{h}")
                    for k in range(K):
                        nc.tensor.matmul(gh_t[:], lm2[k], rhs[h][k],
                                         start=(k == 0), stop=False)
                    nc.tensor.matmul(gh_t[:], lhs5, rhs5[h],
                                     start=False, stop=True)
                    nc.scalar.activation(dn[:, h * NH:(h + 1) * NH],
                                         gh_t[:], Act.Sqrt,
                                         bias=nrma_s[:], scale=1.0)
            else:
                g_n = ppool.tile([R, NB], F32, tag="gn", name="gn")
                for h in (0, 1):
                    gh = g_n[:, h * NH:(h + 1) * NH]
                    for k in range(K):
                        nc.tensor.matmul(gh, lm2[k], rhs[h][k],
                                         start=(k == 0), stop=False)
                    nc.tensor.matmul(gh, lhs5, rhs5[h], start=False,
                                     stop=True)
                nc.scalar.activation(dn[:], g_n[:], Act.Sqrt,
                                     bias=nrma_s[:], scale=1.0)

            # ---- hot loop (separate accum tiles per engine) ----
            fop = _register_fused_op()
            out_t = pool.tile([R, n_dve + 2 * n_act], F32)
            acc_a = pool.tile([R, 2 * max(n_act, 1)], F32)
            if 'no_hot' in abl:
                nc.vector.memset(out_t[:], 1.0)
            for j in (range(0) if 'no_hot' in abl else range(S)):
                pj = pos[:, j:j + 1]
                if j < n_dve:        # custom DVE: fused relu-sum + count
                    z = spool.tile([R, NB], F16, tag="zd", name=f"zd_{j}",
                                   bufs=zd_bufs)
                    nc.vector._custom_dve(
                        fop, out=z[:], in0=dn[:], s0=pj, s1=0.0,
                        imm2=CNT_SCALE, accum_out=out_t[:, j:j + 1])
                else:                # ACT: Relu sum + Sign count on dn
                    # z outs go to PSUM: separate memory from the DVE
                    # streams' SBUF traffic, and ACT's PSUM access is
                    # cheaper than SBUF (172 vs 222 cycles).
                    ja = j - n_dve
                    z1 = ppool.tile([R, NB], F32, tag="za", bufs=2,
                                    name=f"za1_{j}")
                    nc.scalar.activation(z1[:], dn[:], Act.Relu,
                                         bias=pj, scale=-1.0,
                                         accum_out=acc_a[:, 2 * ja:2 * ja + 1])
                    z2 = ppool.tile([R, NB], F32, tag="za", bufs=2,
                                    name=f"za2_{j}")
                    nc.scalar.activation(z2[:], dn[:], Act.Sign,
                                         bias=pj, scale=-1.0,
                                         accum_out=acc_a[:, 2 * ja + 1:2 * ja + 2])
            if n_act and 'no_hot' not in abl:
                (nc.gpsimd if pooloff else nc.vector).tensor_copy(
                    out_t[:, n_dve:n_dve + 2 * n_act], acc_a[:])

            nc.sync.dma_start(out_d.ap(), out_t[:])

        if loop_iters is None:
            _body()
        else:
            with tc.For_i(0, loop_iters, 1):
                _body()

    nc.compile()
    return nc


def _ilv(a, nchunk):
    """[nchunk*128 (contraction), x] -> [128, nchunk*x] chunk-interleaved."""
    x = a.shape[1]
    return np.ascontiguousarray(
        a.reshape(nchunk, 128, x).transpose(1, 0, 2).reshape(128, nchunk * x))


def _prepare(embeddings: np.ndarray, labels: np.ndarray,
             fp8: bool = True, pack: bool = True):
    emb = np.ascontiguousarray(np.asarray(embeddings, dtype=np.float32))
    lab = np.asarray(labels)

    perm = np.argsort(lab, kind="stable")
    e_s = emb[perm]
    lab_s = lab[perm]
    classes, starts, counts = np.unique(lab_s, return_index=True,
                                        return_counts=True)
    C = len(classes)
    S = int(counts.max())
    SLOT = 32 * ((S + 31) // 32)
    spb = -(-C // N_CORES)            # class slots per block
    R = spb * SLOT
    assert R <= 128, f"padded rows per core {R} > 128"

    cls_of_col = np.searchsorted(starts, np.arange(B), side="right") - 1

    # padded anchors: class c -> slot c, rows [SLOT*c, SLOT*c+m_c)
    nP = N_CORES * R
    eP = np.zeros((nP, B), dtype=np.float32)
    cls_of_row = np.repeat(np.arange(-(-nP // SLOT)), SLOT)[:nP]
    live_row = np.zeros((nP,), dtype=bool)
    for c in range(C):
        eP[SLOT * c:SLOT * c + counts[c]] = \
            e_s[starts[c]:starts[c] + counts[c]]
        live_row[SLOT * c:SLOT * c + counts[c]] = True
    ePT = np.ascontiguousarray(eP.T).astype(np.float16)
    e_sT = np.ascontiguousarray(e_s.T).astype(np.float16)  # [512, 512]

    sqa_all = (ePT.astype(np.float32) ** 2).sum(0)         # [nP]
    sqn = (e_sT.astype(np.float32) ** 2).sum(0)            # [512]

    # rhs extra chunk: onehot rows + |b|^2 in row 32
    ch_r = np.zeros((33, NB), dtype=np.float16)
    ch_r[cls_of_col[:NB], np.arange(NB)] = OH
    ch_r[32, :] = sqn.astype(np.float16)

    rhsa = _ilv(e_sT[:, :NH], K)
    rhsb = _ilv(e_sT[:, NH:], K)

    num_valid = float((counts * (counts - 1) * (B - counts)).sum())

    in_maps = []
    for b in range(N_CORES):
        cols = np.arange(R * b, R * b + R)
        livec = live_row[cols]
        row_cls = np.minimum(cls_of_row[cols], C - 1)
        row_m = counts[row_cls]
        # lhs extra chunk: onehot rows (anchor class), ones row
        ch = np.zeros((33, R), dtype=np.float16)
        ch[row_cls, np.arange(R)] = np.where(livec, OH, 0.0)
        ch[32, :] = np.float16(1.0)
        # second extras col-block: row 32 = -2|a|^2
        ch6 = np.zeros((33, R), dtype=np.float16)
        ch6[32, :] = (-2.0 * sqa_all[cols]).astype(np.float16)
        lhs_chunks = ePT[:, cols].reshape(K, 128, R)
        lhsT = np.concatenate([lhs_chunks[k] for k in range(K)], axis=1)
        lm2 = np.ascontiguousarray((-2.0 * lhsT.astype(np.float32))
                                   .astype(np.float16))
        rhsx = np.ascontiguousarray(
            np.concatenate([ch_r, ch, ch6], axis=1))

        ii = np.tile(np.arange(SLOT), spb)
        jj = np.arange(S)[None, :]
        valid = ((jj < row_m[:, None]) & (jj != ii[:, None])
                 & (ii[:, None] < row_m[:, None]))
        mb = np.where(valid, 0.0, NEG).astype(np.float16)
        if fp8:
            import ml_dtypes
            f8 = ml_dtypes.float8_e4m3fn
            lm2 = lm2.astype(np.float32).astype(f8)
            rhsa_c = rhsa.astype(np.float32).astype(f8)
            rhsb_c = rhsb.astype(np.float32).astype(f8)
        else:
            rhsa_c, rhsb_c = rhsa, rhsb
        if pack:
            packA = np.concatenate(
                [np.ascontiguousarray(lm2).view(np.uint8),
                 np.ascontiguousarray(mb).view(np.uint8),
                 np.ascontiguousarray(rhsa_c).view(np.uint8)], axis=1)
            in_maps.append({
                "packA": np.ascontiguousarray(packA),
                "rhsb": rhsb_c,
                "rhsx": rhsx,
            })
        else:
            in_maps.append({
                "lm2": lm2,
                "rhsa": rhsa_c,
                "rhsb": rhsb_c,
                "rhsx": rhsx,
                "mb": mb,
            })
    return S, R, C, in_maps, num_valid


def _combine(outs, num_valid, S, n_act=7):
    n_dve = S - n_act
    loss_sum = 0.0
    num_pos = 0.0
    R = outs[0].shape[0]
    for c in range(N_CORES):
        o = np.asarray(outs[c], dtype=np.float64)
        # fused DVE columns: count = floor(acc/CNT_SCALE), relu = rest
        fused = o[:, 0:n_dve]
        fcnt = np.floor(fused / CNT_SCALE)
        loss_sum += (fused - CNT_SCALE * fcnt).sum()
        num_pos += fcnt.sum()
        # ACT column pairs: +relu sum, sign count
        for ja in range(n_act):
            c0 = n_dve + 2 * ja
            loss_sum += o[:, c0].sum()
            num_pos += 0.5 * (o[:, c0 + 1].sum() + NB * R)
    loss = np.float32(loss_sum / (num_pos + 1e-5))
    frac = np.float32(num_pos / (num_valid + 1e-5))
    return (loss, frac)


def kernel(embeddings: np.ndarray, labels: np.ndarray):
    S, R, C, in_maps, num_valid = _prepare(embeddings, labels)
    n_act = min(7, max(S - 1, 0))
    key = (S, R, C, n_act)
    if key not in _cache:
        _cache[key] = _build(S, R, C, n_act=n_act)
    nc = _cache[key]
    res = run_bass_kernel_spmd(nc, in_maps, core_ids=list(range(N_CORES)))
    return _combine([res.results[c]["out"] for c in range(N_CORES)],
                    num_valid, S, n_act=n_act)


# revision 5
# speedup vs baseline: 1.0595x; 1.0536x over previous
"""BatchAllTripletLoss v5 — generalized group packing, W=18 hot slots.

Anchors may REPLICATE across the 32 (core, group) positions; each 32-row
group gets an arbitrary <=W positive-column window shipped as data (a
dedicated band-gram rhs), so the per-core hot loop is W=18 slots instead
of 24. Classes wider than W split their window across groups (all rows
present in each chunk). The n-side (dn over all 512 negatives) is
unchanged. Pairs are covered exactly once: a class's chunks partition
its columns and every chunk carries all its rows.
"""
import sys
sys.path.insert(0, "/opt/trn_rl_repo")

import numpy as np
from contextlib import ExitStack

import concourse.bass as bass
import concourse.tile as tile
from concourse import bacc, mybir
from concourse.bass_utils import run_bass_kernel_spmd

F32 = mybir.dt.float32
F16 = mybir.dt.float16
Alu = mybir.AluOpType
Act = mybir.ActivationFunctionType

B = 512
K = 4
NB = 512
NH = 256
N_CORES = 8
OH = 1000.0
NEG = -3032.0
W = 18              # hot slots (band window per 32-row group)
NGRP = 4            # 32-row groups per core

_cache = {}
CNT_SCALE = 8192.0


def _register_fused_op():
    import concourse.dve_ops as dve_ops
    from concourse.dve_spec import Spec, Src0, C0, C1, C2, relu, lower, \
        _has_src1
    from concourse.dve_ops import DveOp
    from concourse.dve_uop import DveOpSpec
    name = "RELU_RSUB_CNT_ANT"
    if name in dve_ops._SUB_OPCODE_FOR_NAME:
        for o in dve_ops.OPS:
            if o.name == name:
                return o
    spec = Spec(
        body=relu(C0 - Src0) + ((C0 - Src0) > C1) * C2,
        accum=__import__("operator").add,
        reference=lambda in0, s0, s1, imm2: (
            np.maximum(s0 - in0.astype(np.float32), 0.0)
            + imm2 * ((s0 - in0.astype(np.float32)) > s1)),
    )
    op = DveOp(name, spec, subdim=False, uops_sha={})
    row = dve_ops._CUSTOM_DVE_ROW_BASE + len(dve_ops.OPS)
    assert row < 0x20
    dve_ops.OPS.append(op)
    dve_ops.CUSTOM_DVE_SPECS[name] = spec
    dve_ops._SUB_OPCODE_FOR_NAME[name] = row
    for ver in ("v3", "v4"):
        s = DveOpSpec(name=name, opcode=row, uops=lower(spec, ver=ver),
                      rd1_en=_has_src1(spec))
        object.__setattr__(op, "uops_sha", {**op.uops_sha, ver: s.sha(ver)})
    return op


def _build(S: int, R: int, C: int, n_act: int = 6, n_warm: int = 7,
           loop_iters: int | None = None):
    """S here = W (hot slots per group window)."""
    n_dve = S - n_act
    assert n_dve >= 0 and R == 128

    nc = bacc.Bacc("TRN2", target_bir_lowering=False, debug=False,
                   num_devices=N_CORES)

    FE = mybir.dt.float8e4
    U8 = mybir.dt.uint8
    BW = NGRP * S          # band-gram columns (4 groups x W)
    # packA: [lm2 | mb | band_rhs | rhsa] bytes on the sync queue
    PW = K * R + 2 * S * R // R * 1 * 2 + K * BW + K * NH  # fp8 sizes + mb f16
    PW = K * R * 1 + 2 * S + K * BW * 1 + K * NH * 1
    packA_d = nc.dram_tensor("packA", [128, PW], U8, kind="ExternalInput")
    rhsb_d = nc.dram_tensor("rhsb", [128, K * NH], FE, kind="ExternalInput")
    # rhsx: [rhs-extras(NB) | lhs-onehot(R) | lhs-nrm(R) | band-nrm(BW)]
    rhsx_d = nc.dram_tensor("rhsx", [33, NB + 2 * R + BW], F16,
                            kind="ExternalInput")
    out_d = nc.dram_tensor("out", [R, S + n_act], F32,
                           kind="ExternalOutput")

    with tile.TileContext(nc) as tc, ExitStack() as ctx:
        pool = ctx.enter_context(tc.tile_pool(name="sbuf", bufs=2))
        spool = ctx.enter_context(tc.tile_pool(name="scr", bufs=3))
        ppool = ctx.enter_context(tc.tile_pool(name="psum", bufs=2,
                                               space="PSUM"))

        def _body():
            rhsb_t = pool.tile([128, K, NH], FE)
            rhsx_t = pool.tile([33, NB + 2 * R + BW], F16)
            packA_t = pool.tile([128, PW], U8)
            nc.sync.dma_start(packA_t[:], packA_d.ap())
            o1 = K * R
            o2 = o1 + 2 * S
            o3 = o2 + K * BW
            lm2_t = packA_t[:, 0:o1].bitcast(FE)
            mb_t = packA_t[0:R, o1:o2].bitcast(F16)
            brhs_flat = packA_t[:, o2:o3].bitcast(FE)
            rhsa_flat = packA_t[:, o3:PW].bitcast(FE)
            nc.scalar.dma_start(rhsx_t[:], rhsx_d.ap())
            nc.scalar.dma_start(rhsb_t[:], rhsb_d.ap())

            lm2 = [lm2_t[:, k * R:(k + 1) * R] for k in range(K)]
            brhs = [brhs_flat[:, k * BW:(k + 1) * BW] for k in range(K)]
            rhs = {0: [rhsa_flat[:, k * NH:(k + 1) * NH] for k in range(K)],
                   1: [rhsb_t[:, k, :] for k in range(K)]}
            lhs5 = rhsx_t[0:33, NB:NB + R]
            lhs5_ones = rhsx_t[32:33, NB:NB + R]
            lhs6_nrm = rhsx_t[32:33, NB + R:NB + 2 * R]      # -2|a|^2
            bnrm = rhsx_t[32:33, NB + 2 * R:NB + 2 * R + BW]  # |p|^2
            rhs5 = {0: rhsx_t[0:33, 0:NH], 1: rhsx_t[0:33, NH:NB]}

            one16 = pool.tile([33, 1], F16)
            dumt = pool.tile([1, 2], F32)
            nc.vector.memset(one16[:], 1.0)
            nc.vector.memset(dumt[:], 1.0)
            dums = pool.tile([1, 2], F32)
            nc.scalar.activation(dums[:], dumt[:], Act.Sqrt)
            if n_warm:
                wsrc = pool.tile([128, NB], F16)
                nc.gpsimd.memset(wsrc[:], 0.0)
                warm = ppool.tile([1, NB], F32, tag="warm", name="warm",
                                  bufs=1)
                for w in range(n_warm):
                    nc.tensor.matmul(warm[:], wsrc[:, 0:1], wsrc[:],
                                     start=True, stop=True)

            # anchor-norm column from -2|a|^2 row
            nrma = ppool.tile([R, 1], F32, tag="nrma", name="nrma", bufs=1)
            nc.tensor.matmul(nrma[:], lhs6_nrm, one16[32:33, :],
                             start=True, stop=True)
            nrma_s = pool.tile([R, 1], F32)
            nc.vector.tensor_scalar(out=nrma_s[:], in0=nrma[:],
                                    scalar1=-0.5, scalar2=None, op0=Alu.mult)

            # ---- band gram: -2 a^T p + |p|^2  [R, 4*W] ----
            g_b = ppool.tile([R, BW], F32, tag="gb", name="gb")
            for k in range(K):
                nc.tensor.matmul(g_b[:], lm2[k], brhs[k],
                                 start=(k == 0), stop=False)
            nc.tensor.matmul(g_b[:], lhs5_ones, bnrm, start=False, stop=True)
            band2 = pool.tile([R, S], F32)
            for g in range(NGRP):
                nc.scalar.activation(
                    band2[g * 32:(g + 1) * 32, :],
                    g_b[g * 32:(g + 1) * 32, g * S:(g + 1) * S],
                    Act.Relu, bias=nrma_s[g * 32:(g + 1) * 32, :],
                    scale=1.0)
            dband = pool.tile([R, S], F32)
            nc.scalar.activation(dband[:], band2[:], Act.Sqrt)
            pos = pool.tile([R, S], F32)
            nc.vector.tensor_tensor(out=pos[:], in0=dband[:], in1=mb_t,
                                    op=Alu.add)

            # ---- n-side gram + dn (unchanged) ----
            g_n = ppool.tile([R, NB], F32, tag="gn", name="gn")
            for h in (0, 1):
                gh = g_n[:, h * NH:(h + 1) * NH]
                for k in range(K):
                    nc.tensor.matmul(gh, lm2[k], rhs[h][k],
                                     start=(k == 0), stop=False)
                nc.tensor.matmul(gh, lhs5, rhs5[h], start=False, stop=True)
            dn = pool.tile([R, NB], F16)
            nc.scalar.activation(dn[:], g_n[:], Act.Sqrt,
                                 bias=nrma_s[:], scale=1.0)

            # ---- hot loop ----
            fop = _register_fused_op()
            out_t = pool.tile([R, n_dve + 2 * n_act], F32)
            acc_a = pool.tile([R, 2 * max(n_act, 1)], F32)
            for j in range(S):
                pj = pos[:, j:j + 1]
                if j < n_dve:
                    z = spool.tile([R, NB], F16, tag="zd", name=f"zd_{j}")
                    nc.vector._custom_dve(
                        fop, out=z[:], in0=dn[:], s0=pj, s1=0.0,
                        imm2=CNT_SCALE, accum_out=out_t[:, j:j + 1])
                else:
                    ja = j - n_dve
                    z1 = ppool.tile([R, NB], F32, tag="za", bufs=2,
                                    name=f"za1_{j}")
                    nc.scalar.activation(z1[:], dn[:], Act.Relu,
                                         bias=pj, scale=-1.0,
                                         accum_out=acc_a[:, 2 * ja:2 * ja + 1])
                    z2 = ppool.tile([R, NB], F32, tag="za", bufs=2,
                                    name=f"za2_{j}")
                    nc.scalar.activation(z2[:], dn[:], Act.Sign,
                                         bias=pj, scale=-1.0,
                                         accum_out=acc_a[:, 2 * ja + 1:2 * ja + 2])
            if n_act:
                nc.vector.tensor_copy(
                    out_t[:, n_dve:n_dve + 2 * n_act], acc_a[:])
            nc.sync.dma_start(out_d.ap(), out_t[:])

        if loop_iters is None:
            _body()
        else:
            with tc.For_i(0, loop_iters, 1):
                _body()

    nc.compile()
    return nc


def _ilv(a, nchunk):
    x = a.shape[1]
    return np.ascontiguousarray(
        a.reshape(nchunk, 128, x).transpose(1, 0, 2).reshape(128, nchunk * x))


def _pack_bins(counts):
    """Pack class windows into 32 bins: rows(bin) = sum of member classes'
    counts <= 32; cols(bin) = sum of member chunk widths <= W. Big classes
    appear in multiple bins with disjoint column slices (all rows each)."""
    rem = {cid: int(c) for cid, c in enumerate(counts)}
    cnt = dict(rem)
    bins = []
    for _ in range(N_CORES * NGRP):
        rows = cols = 0
        members = []
        progress = True
        while progress and cols < W:
            progress = False
            for cid, c in sorted(rem.items(), key=lambda x: -x[1]):
                if c == 0:
                    continue
                if c <= W - cols and rows + cnt[cid] <= 32:
                    members.append((cid, 0 if c == cnt[cid] else
                                    cnt[cid] - rem[cid], c))
                    rows += cnt[cid]
                    cols += c
                    rem[cid] = 0
                    progress = True
                    break
            if progress:
                continue
            for cid, c in sorted(rem.items(), key=lambda x: -x[1]):
                if c == 0:
                    continue
                take = min(c, W - cols)
                if take > 0 and rows + cnt[cid] <= 32:
                    members.append((cid, cnt[cid] - rem[cid], take))
                    rows += cnt[cid]
                    cols += take
                    rem[cid] -= take
                    progress = True
                    break
        bins.append(members)
    assert all(v == 0 for v in rem.values()), "packing failed"
    return bins


def _prepare(embeddings: np.ndarray, labels: np.ndarray):
    emb = np.ascontiguousarray(np.asarray(embeddings, dtype=np.float32))
    lab = np.asarray(labels)
    import ml_dtypes
    f8 = ml_dtypes.float8_e4m3fn

    perm = np.argsort(lab, kind="stable")
    e_s = emb[perm]
    lab_s = np.asarray(lab)[perm]
    classes, starts, counts = np.unique(lab_s, return_index=True,
                                        return_counts=True)
    C = len(classes)
    R = 128
    bins = _pack_bins(counts)

    cls_of_col = np.searchsorted(starts, np.arange(B), side="right") - 1
    e_sT = np.ascontiguousarray(e_s.T).astype(np.float32)   # [512, 512]
    sqn = (e_sT.astype(np.float16).astype(np.float32) ** 2).sum(0)

    # rhs extras (negatives): onehot + |b|^2 (same for all cores)
    ch_r = np.zeros((33, NB), dtype=np.float16)
    ch_r[cls_of_col[:NB], np.arange(NB)] = OH
    ch_r[32, :] = sqn.astype(np.float16)
    e8 = e_sT.astype(np.float16).astype(np.float32)
    rhsa = _ilv(e8[:, :NH].astype(f8), K)
    rhsb = _ilv(e8[:, NH:].astype(f8), K)

    num_valid = float((counts * (counts - 1) * (B - counts)).sum())
    BW = NGRP * W

    in_maps = []
    for b in range(N_CORES):
        core_bins = bins[NGRP * b:NGRP * b + NGRP]
        # anchors: 4 groups x 32 row-slots
        anchors = np.zeros(R, dtype=np.int64)      # col index into e_s
        live = np.zeros(R, dtype=bool)
        acls = np.full(R, -1, dtype=np.int64)
        bcols = np.zeros(BW, dtype=np.int64)
        bcls = np.full(BW, -1, dtype=np.int64)
        bliv = np.zeros(BW, dtype=bool)
        for g, members in enumerate(core_bins):
            r0 = 32 * g
            c0 = W * g
            ri = ci = 0
            for cid, coff, ncols in members:
                s0 = int(starts[cid])
                cn = int(counts[cid])
                anchors[r0 + ri:r0 + ri + cn] = np.arange(s0, s0 + cn)
                live[r0 + ri:r0 + ri + cn] = True
                acls[r0 + ri:r0 + ri + cn] = cid
                ri += cn
                bcols[c0 + ci:c0 + ci + ncols] = np.arange(
                    s0 + coff, s0 + coff + ncols)
                bcls[c0 + ci:c0 + ci + ncols] = cid
                bliv[c0 + ci:c0 + ci + ncols] = True
                ci += ncols
        eA = np.where(live[None, :], e8[:, anchors], 0.0)   # [512, 128]
        lm2 = _ilv((-2.0 * eA).astype(f8), K)               # [128, K*R]
        brhs = _ilv(np.where(bliv[None, :], e8[:, bcols], 0.0).astype(f8),
                    K)                                      # [128, K*BW]
        sqa = np.where(live, (eA ** 2).sum(0), 0.0)
        sqp = np.where(bliv, (e8[:, bcols] ** 2).sum(0), 0.0)

        ch = np.zeros((33, R), dtype=np.float16)            # lhs onehot
        ch[np.maximum(acls, 0), np.arange(R)] = np.where(live, OH, 0.0)
        ch[32, :] = np.float16(1.0)
        ch6 = np.zeros((33, R), dtype=np.float16)
        ch6[32, :] = (-2.0 * sqa).astype(np.float16)
        chb = np.zeros((33, BW), dtype=np.float16)          # band |p|^2
        chb[32, :] = sqp.astype(np.float16)
        rhsx = np.ascontiguousarray(
            np.concatenate([ch_r, ch, ch6, chb], axis=1))

        # mb: valid iff same class, anchor != col, both live
        gi = np.arange(R) // 32
        mb = np.full((R, W), NEG, dtype=np.float16)
        for r in range(R):
            if not live[r]:
                continue
            cw = slice(W * gi[r], W * gi[r] + W)
            ok = (bcls[cw] == acls[r]) & bliv[cw] & \
                 (bcols[cw] != anchors[r])
            mb[r, :] = np.where(ok, 0.0, NEG)

        mbu8 = np.ascontiguousarray(mb).view(np.uint8)      # [128, 2W]
        packA = np.concatenate(
            [np.ascontiguousarray(lm2).view(np.uint8), mbu8,
             np.ascontiguousarray(brhs).view(np.uint8),
             np.ascontiguousarray(rhsa).view(np.uint8)], axis=1)
        in_maps.append({
            "packA": np.ascontiguousarray(packA),
            "rhsb": rhsb,
            "rhsx": rhsx,
        })
    return W, R, C, in_maps, num_valid


def _combine(outs, num_valid, S, n_act=6):
    n_dve = S - n_act
    loss_sum = 0.0
    num_pos = 0.0
    R = outs[0].shape[0]
    for c in range(N_CORES):
        o = np.asarray(outs[c], dtype=np.float64)
        fused = o[:, 0:n_dve]
        fcnt = np.floor(fused / CNT_SCALE)
        loss_sum += (fused - CNT_SCALE * fcnt).sum()
        num_pos += fcnt.sum()
        for ja in range(n_act):
            c0 = n_dve + 2 * ja
            loss_sum += o[:, c0].sum()
            num_pos += 0.5 * (o[:, c0 + 1].sum() + NB * R)
    loss = np.float32(loss_sum / (num_pos + 1e-5))
    frac = np.float32(num_pos / (num_valid + 1e-5))
    return (loss, frac)


def kernel(embeddings: np.ndarray, labels: np.ndarray):
    S, R, C, in_maps, num_valid = _prepare(embeddings, labels)
    key = (S, R, C)
    if key not in _cache:
        _cache[key] = _build(S, R, C)
    nc = _cache[key]
    res = run_bass_kernel_spmd(nc, in_maps, core_ids=list(range(N_CORES)))
    return _combine([res.results[c]["out"] for c in range(N_CORES)],
                    num_valid, S)


# revision 6
# speedup vs baseline: 1.1643x; 1.0990x over previous
"""BatchAllTripletLoss v5 — generalized group packing, W=18 hot slots.

Anchors may REPLICATE across the 32 (core, group) positions; each 32-row
group gets an arbitrary <=W positive-column window shipped as data (a
dedicated band-gram rhs), so the per-core hot loop is W=18 slots instead
of 24. Classes wider than W split their window across groups (all rows
present in each chunk). The n-side (dn over all 512 negatives) is
unchanged. Pairs are covered exactly once: a class's chunks partition
its columns and every chunk carries all its rows.
"""
import sys
sys.path.insert(0, "/opt/trn_rl_repo")

import numpy as np
from contextlib import ExitStack

import concourse.bass as bass
import concourse.tile as tile
from concourse import bacc, mybir
from concourse.bass_utils import run_bass_kernel_spmd

F32 = mybir.dt.float32
F16 = mybir.dt.float16
Alu = mybir.AluOpType
Act = mybir.ActivationFunctionType

B = 512
K = 4
NB = 512
NH = 256
N_CORES = 8
OH = 1000.0
NEG = -3032.0
W = 17              # hot slots (band window per 32-row group)
NGRP = 4            # 32-row groups per core

_cache = {}
CNT_SCALE = 8192.0


def _register_fused_op():
    import concourse.dve_ops as dve_ops
    from concourse.dve_spec import Spec, Src0, C0, C1, C2, relu, lower, \
        _has_src1
    from concourse.dve_ops import DveOp
    from concourse.dve_uop import DveOpSpec
    name = "RELU_RSUB_CNT_ANT"
    if name in dve_ops._SUB_OPCODE_FOR_NAME:
        for o in dve_ops.OPS:
            if o.name == name:
                return o
    spec = Spec(
        body=relu(C0 - Src0) + ((C0 - Src0) > C1) * C2,
        accum=__import__("operator").add,
        reference=lambda in0, s0, s1, imm2: (
            np.maximum(s0 - in0.astype(np.float32), 0.0)
            + imm2 * ((s0 - in0.astype(np.float32)) > s1)),
    )
    op = DveOp(name, spec, subdim=False, uops_sha={})
    row = dve_ops._CUSTOM_DVE_ROW_BASE + len(dve_ops.OPS)
    assert row < 0x20
    dve_ops.OPS.append(op)
    dve_ops.CUSTOM_DVE_SPECS[name] = spec
    dve_ops._SUB_OPCODE_FOR_NAME[name] = row
    for ver in ("v3", "v4"):
        s = DveOpSpec(name=name, opcode=row, uops=lower(spec, ver=ver),
                      rd1_en=_has_src1(spec))
        object.__setattr__(op, "uops_sha", {**op.uops_sha, ver: s.sha(ver)})
    return op


def _build(S: int, R: int, C: int, n_act: int = 5, n_warm: int = 7,
           loop_iters: int | None = None):
    """S here = W (hot slots per group window)."""
    n_dve = S - n_act
    assert n_dve >= 0 and R == 128

    nc = bacc.Bacc("TRN2", target_bir_lowering=False, debug=False,
                   num_devices=N_CORES)

    FE = mybir.dt.float8e4
    U8 = mybir.dt.uint8
    BW = NGRP * S          # band-gram columns (4 groups x W)
    # packA: [lm2 | mb | band_rhs | rhsa] bytes on the sync queue
    PW = K * R + 2 * S * R // R * 1 * 2 + K * BW + K * NH  # fp8 sizes + mb f16
    PW = K * R * 1 + 2 * S + K * BW * 1 + K * NH * 1
    packA_d = nc.dram_tensor("packA", [128, PW], U8, kind="ExternalInput")
    rhsb_d = nc.dram_tensor("rhsb", [128, K * NH], FE, kind="ExternalInput")
    # rhsx: [rhs-extras(NB) | lhs-onehot(R) | lhs-nrm(R) | band-nrm(BW)]
    rhsx_d = nc.dram_tensor("rhsx", [33, NB + 2 * R + BW], F16,
                            kind="ExternalInput")
    out_d = nc.dram_tensor("out", [R, S + n_act], F32,
                           kind="ExternalOutput")

    with tile.TileContext(nc) as tc, ExitStack() as ctx:
        pool = ctx.enter_context(tc.tile_pool(name="sbuf", bufs=2))
        spool = ctx.enter_context(tc.tile_pool(name="scr", bufs=3))
        ppool = ctx.enter_context(tc.tile_pool(name="psum", bufs=2,
                                               space="PSUM"))

        def _body():
            rhsb_t = pool.tile([128, K, NH], FE)
            rhsx_t = pool.tile([33, NB + 2 * R + BW], F16)
            packA_t = pool.tile([128, PW], U8)
            nc.sync.dma_start(packA_t[:], packA_d.ap())
            o1 = K * R
            o2 = o1 + 2 * S
            o3 = o2 + K * BW
            lm2_t = packA_t[:, 0:o1].bitcast(FE)
            mb_t = packA_t[0:R, o1:o2].bitcast(F16)
            brhs_flat = packA_t[:, o2:o3].bitcast(FE)
            rhsa_flat = packA_t[:, o3:PW].bitcast(FE)
            nc.scalar.dma_start(rhsx_t[:], rhsx_d.ap())
            nc.scalar.dma_start(rhsb_t[:], rhsb_d.ap())

            lm2 = [lm2_t[:, k * R:(k + 1) * R] for k in range(K)]
            brhs = [brhs_flat[:, k * BW:(k + 1) * BW] for k in range(K)]
            rhs = {0: [rhsa_flat[:, k * NH:(k + 1) * NH] for k in range(K)],
                   1: [rhsb_t[:, k, :] for k in range(K)]}
            lhs5 = rhsx_t[0:33, NB:NB + R]
            lhs5_ones = rhsx_t[32:33, NB:NB + R]
            lhs6_nrm = rhsx_t[32:33, NB + R:NB + 2 * R]      # -2|a|^2
            bnrm = rhsx_t[32:33, NB + 2 * R:NB + 2 * R + BW]  # |p|^2
            rhs5 = {0: rhsx_t[0:33, 0:NH], 1: rhsx_t[0:33, NH:NB]}

            one16 = pool.tile([33, 1], F16)
            dumt = pool.tile([1, 2], F32)
            nc.vector.memset(one16[:], 1.0)
            nc.vector.memset(dumt[:], 1.0)
            dums = pool.tile([1, 2], F32)
            nc.scalar.activation(dums[:], dumt[:], Act.Sqrt)
            if n_warm:
                wsrc = pool.tile([128, NB], F16)
                nc.gpsimd.memset(wsrc[:], 0.0)
                warm = ppool.tile([1, NB], F32, tag="warm", name="warm",
                                  bufs=1)
                for w in range(n_warm):
                    nc.tensor.matmul(warm[:], wsrc[:, 0:1], wsrc[:],
                                     start=True, stop=True)

            # anchor-norm column from -2|a|^2 row
            nrma = ppool.tile([R, 1], F32, tag="nrma", name="nrma", bufs=1)
            nc.tensor.matmul(nrma[:], lhs6_nrm, one16[32:33, :],
                             start=True, stop=True)
            nrma_s = pool.tile([R, 1], F32)
            nc.vector.tensor_scalar(out=nrma_s[:], in0=nrma[:],
                                    scalar1=-0.5, scalar2=None, op0=Alu.mult)

            # ---- band gram: -2 a^T p + |p|^2  [R, 4*W] ----
            g_b = ppool.tile([R, BW], F32, tag="gb", name="gb")
            for k in range(K):
                nc.tensor.matmul(g_b[:], lm2[k], brhs[k],
                                 start=(k == 0), stop=False)
            nc.tensor.matmul(g_b[:], lhs5_ones, bnrm, start=False, stop=True)
            band2 = pool.tile([R, S], F32)
            for g in range(NGRP):
                nc.scalar.activation(
                    band2[g * 32:(g + 1) * 32, :],
                    g_b[g * 32:(g + 1) * 32, g * S:(g + 1) * S],
                    Act.Relu, bias=nrma_s[g * 32:(g + 1) * 32, :],
                    scale=1.0)
            dband = pool.tile([R, S], F32)
            nc.scalar.activation(dband[:], band2[:], Act.Sqrt)
            pos = pool.tile([R, S], F32)
            nc.vector.tensor_tensor(out=pos[:], in0=dband[:], in1=mb_t,
                                    op=Alu.add)

            # ---- n-side gram + dn (unchanged) ----
            g_n = ppool.tile([R, NB], F32, tag="gn", name="gn")
            for h in (0, 1):
                gh = g_n[:, h * NH:(h + 1) * NH]
                for k in range(K):
                    nc.tensor.matmul(gh, lm2[k], rhs[h][k],
                                     start=(k == 0), stop=False)
                nc.tensor.matmul(gh, lhs5, rhs5[h], start=False, stop=True)
            dn = pool.tile([R, NB], F16)
            nc.scalar.activation(dn[:], g_n[:], Act.Sqrt,
                                 bias=nrma_s[:], scale=1.0)

            # ---- hot loop ----
            fop = _register_fused_op()
            out_t = pool.tile([R, n_dve + 2 * n_act], F32)
            acc_a = pool.tile([R, 2 * max(n_act, 1)], F32)
            for j in range(S):
                pj = pos[:, j:j + 1]
                if j < n_dve:
                    z = spool.tile([R, NB], F16, tag="zd", name=f"zd_{j}")
                    nc.vector._custom_dve(
                        fop, out=z[:], in0=dn[:], s0=pj, s1=0.0,
                        imm2=CNT_SCALE, accum_out=out_t[:, j:j + 1])
                else:
                    ja = j - n_dve
                    z1 = ppool.tile([R, NB], F32, tag="za", bufs=2,
                                    name=f"za1_{j}")
                    nc.scalar.activation(z1[:], dn[:], Act.Relu,
                                         bias=pj, scale=-1.0,
                                         accum_out=acc_a[:, 2 * ja:2 * ja + 1])
                    z2 = ppool.tile([R, NB], F32, tag="za", bufs=2,
                                    name=f"za2_{j}")
                    nc.scalar.activation(z2[:], dn[:], Act.Sign,
                                         bias=pj, scale=-1.0,
                                         accum_out=acc_a[:, 2 * ja + 1:2 * ja + 2])
            if n_act:
                nc.vector.tensor_copy(
                    out_t[:, n_dve:n_dve + 2 * n_act], acc_a[:])
            nc.sync.dma_start(out_d.ap(), out_t[:])

        if loop_iters is None:
            _body()
        else:
            with tc.For_i(0, loop_iters, 1):
                _body()

    nc.compile()
    return nc


def _ilv(a, nchunk):
    x = a.shape[1]
    return np.ascontiguousarray(
        a.reshape(nchunk, 128, x).transpose(1, 0, 2).reshape(128, nchunk * x))


def _pack_bins(counts):
    """Pack class windows into 32 bins: rows(bin) = sum of member classes'
    counts <= 32; cols(bin) = sum of member chunk widths <= W. Big classes
    appear in multiple bins with disjoint column slices (all rows each).
    Randomized-restart greedy; deterministic (seeds tried in order)."""
    import random

    def attempt(seed):
        rng = random.Random(seed)
        rem = {cid: int(c) for cid, c in enumerate(counts)}
        cnt = dict(rem)
        bins = []
        for _ in range(N_CORES * NGRP):
            rows = cols = 0
            members = []
            while cols < W:
                placed = False
                cands = [(cid, c) for cid, c in rem.items() if c > 0]
                if not cands:
                    break
                rng.shuffle(cands)
                cands.sort(key=lambda x: (-x[1] + rng.random() * 2))
                for cid, c in cands:
                    if c <= W - cols and rows + cnt[cid] <= 32:
                        members.append((cid, cnt[cid] - rem[cid], c))
                        rows += cnt[cid]
                        cols += c
                        rem[cid] = 0
                        placed = True
                        break
                if placed:
                    continue
                for cid, c in cands:
                    take = min(c, W - cols)
                    if take > 0 and rows + cnt[cid] <= 32:
                        members.append((cid, cnt[cid] - rem[cid], take))
                        rows += cnt[cid]
                        cols += take
                        rem[cid] -= take
                        placed = True
                        break
                if not placed:
                    break
            bins.append(members)
        return bins, sum(rem.values())

    for seed in range(50000):
        bins, left = attempt(seed)
        if left == 0:
            return bins
    raise AssertionError("packing failed")


def _prepare(embeddings: np.ndarray, labels: np.ndarray):
    emb = np.ascontiguousarray(np.asarray(embeddings, dtype=np.float32))
    lab = np.asarray(labels)
    import ml_dtypes
    f8 = ml_dtypes.float8_e4m3fn

    perm = np.argsort(lab, kind="stable")
    e_s = emb[perm]
    lab_s = np.asarray(lab)[perm]
    classes, starts, counts = np.unique(lab_s, return_index=True,
                                        return_counts=True)
    C = len(classes)
    R = 128
    bins = _pack_bins(counts)

    cls_of_col = np.searchsorted(starts, np.arange(B), side="right") - 1
    e_sT = np.ascontiguousarray(e_s.T).astype(np.float32)   # [512, 512]
    sqn = (e_sT.astype(np.float16).astype(np.float32) ** 2).sum(0)

    # rhs extras (negatives): onehot + |b|^2 (same for all cores)
    ch_r = np.zeros((33, NB), dtype=np.float16)
    ch_r[cls_of_col[:NB], np.arange(NB)] = OH
    ch_r[32, :] = sqn.astype(np.float16)
    e8 = e_sT.astype(np.float16).astype(np.float32)
    rhsa = _ilv(e8[:, :NH].astype(f8), K)
    rhsb = _ilv(e8[:, NH:].astype(f8), K)

    num_valid = float((counts * (counts - 1) * (B - counts)).sum())
    BW = NGRP * W

    in_maps = []
    for b in range(N_CORES):
        core_bins = bins[NGRP * b:NGRP * b + NGRP]
        # anchors: 4 groups x 32 row-slots
        anchors = np.zeros(R, dtype=np.int64)      # col index into e_s
        live = np.zeros(R, dtype=bool)
        acls = np.full(R, -1, dtype=np.int64)
        bcols = np.zeros(BW, dtype=np.int64)
        bcls = np.full(BW, -1, dtype=np.int64)
        bliv = np.zeros(BW, dtype=bool)
        for g, members in enumerate(core_bins):
            r0 = 32 * g
            c0 = W * g
            ri = ci = 0
            for cid, coff, ncols in members:
                s0 = int(starts[cid])
                cn = int(counts[cid])
                anchors[r0 + ri:r0 + ri + cn] = np.arange(s0, s0 + cn)
                live[r0 + ri:r0 + ri + cn] = True
                acls[r0 + ri:r0 + ri + cn] = cid
                ri += cn
                bcols[c0 + ci:c0 + ci + ncols] = np.arange(
                    s0 + coff, s0 + coff + ncols)
                bcls[c0 + ci:c0 + ci + ncols] = cid
                bliv[c0 + ci:c0 + ci + ncols] = True
                ci += ncols
        eA = np.where(live[None, :], e8[:, anchors], 0.0)   # [512, 128]
        lm2 = _ilv((-2.0 * eA).astype(f8), K)               # [128, K*R]
        brhs = _ilv(np.where(bliv[None, :], e8[:, bcols], 0.0).astype(f8),
                    K)                                      # [128, K*BW]
        sqa = np.where(live, (eA ** 2).sum(0), 0.0)
        sqp = np.where(bliv, (e8[:, bcols] ** 2).sum(0), 0.0)

        ch = np.zeros((33, R), dtype=np.float16)            # lhs onehot
        ch[np.maximum(acls, 0), np.arange(R)] = np.where(live, OH, 0.0)
        ch[32, :] = np.float16(1.0)
        ch6 = np.zeros((33, R), dtype=np.float16)
        ch6[32, :] = (-2.0 * sqa).astype(np.float16)
        chb = np.zeros((33, BW), dtype=np.float16)          # band |p|^2
        chb[32, :] = sqp.astype(np.float16)
        rhsx = np.ascontiguousarray(
            np.concatenate([ch_r, ch, ch6, chb], axis=1))

        # mb: valid iff same class, anchor != col, both live
        gi = np.arange(R) // 32
        mb = np.full((R, W), NEG, dtype=np.float16)
        for r in range(R):
            if not live[r]:
                continue
            cw = slice(W * gi[r], W * gi[r] + W)
            ok = (bcls[cw] == acls[r]) & bliv[cw] & \
                 (bcols[cw] != anchors[r])
            mb[r, :] = np.where(ok, 0.0, NEG)

        mbu8 = np.ascontiguousarray(mb).view(np.uint8)      # [128, 2W]
        packA = np.concatenate(
            [np.ascontiguousarray(lm2).view(np.uint8), mbu8,
             np.ascontiguousarray(brhs).view(np.uint8),
             np.ascontiguousarray(rhsa).view(np.uint8)], axis=1)
        in_maps.append({
            "packA": np.ascontiguousarray(packA),
            "rhsb": rhsb,
            "rhsx": rhsx,
        })
    return W, R, C, in_maps, num_valid


def _combine(outs, num_valid, S, n_act=5):
    n_dve = S - n_act
    loss_sum = 0.0
    num_pos = 0.0
    R = outs[0].shape[0]
    for c in range(N_CORES):
        o = np.asarray(outs[c], dtype=np.float64)
        fused = o[:, 0:n_dve]
        fcnt = np.floor(fused / CNT_SCALE)
        loss_sum += (fused - CNT_SCALE * fcnt).sum()
        num_pos += fcnt.sum()
        for ja in range(n_act):
            c0 = n_dve + 2 * ja
            loss_sum += o[:, c0].sum()
            num_pos += 0.5 * (o[:, c0 + 1].sum() + NB * R)
    loss = np.float32(loss_sum / (num_pos + 1e-5))
    frac = np.float32(num_pos / (num_valid + 1e-5))
    return (loss, frac)


def kernel(embeddings: np.ndarray, labels: np.ndarray):
    S, R, C, in_maps, num_valid = _prepare(embeddings, labels)
    key = (S, R, C)
    if key not in _cache:
        _cache[key] = _build(S, R, C)
    nc = _cache[key]
    res = run_bass_kernel_spmd(nc, in_maps, core_ids=list(range(N_CORES)))
    return _combine([res.results[c]["out"] for c in range(N_CORES)],
                    num_valid, S)


# revision 7
# speedup vs baseline: 1.1702x; 1.0050x over previous
"""BatchAllTripletLoss v5 — generalized group packing, W=17 hot slots.

Anchors may REPLICATE across the 32 (core, group) positions; each 32-row
group gets an arbitrary <=W positive-column window shipped as data (a
dedicated band-gram rhs), so the per-core hot loop is W=17 slots instead
of 24 (W=16 is provably infeasible: zero column slack; the 23/24-count
classes force wasted slots, and row-splitting a class doubles its
column usage). Classes wider than W split their window across groups (all rows
present in each chunk). The n-side (dn over all 512 negatives) is
unchanged. Pairs are covered exactly once: a class's chunks partition
its columns and every chunk carries all its rows.
"""
import sys
sys.path.insert(0, "/opt/trn_rl_repo")

import numpy as np
from contextlib import ExitStack

import concourse.bass as bass
import concourse.tile as tile
from concourse import bacc, mybir
from concourse.bass_utils import run_bass_kernel_spmd

F32 = mybir.dt.float32
F16 = mybir.dt.float16
Alu = mybir.AluOpType
Act = mybir.ActivationFunctionType

B = 512
K = 4
NB = 512
NH = 256
N_CORES = 8
OH = 1000.0
NEG = -3032.0
W = 17              # hot slots (band window per 32-row group)
NGRP = 4            # 32-row groups per core

_cache = {}
CNT_SCALE = 8192.0


def _register_fused_op():
    import concourse.dve_ops as dve_ops
    from concourse.dve_spec import Spec, Src0, C0, C1, C2, relu, lower, \
        _has_src1
    from concourse.dve_ops import DveOp
    from concourse.dve_uop import DveOpSpec
    name = "RELU_RSUB_CNT_ANT"
    if name in dve_ops._SUB_OPCODE_FOR_NAME:
        for o in dve_ops.OPS:
            if o.name == name:
                return o
    spec = Spec(
        body=relu(C0 - Src0) + ((C0 - Src0) > C1) * C2,
        accum=__import__("operator").add,
        reference=lambda in0, s0, s1, imm2: (
            np.maximum(s0 - in0.astype(np.float32), 0.0)
            + imm2 * ((s0 - in0.astype(np.float32)) > s1)),
    )
    op = DveOp(name, spec, subdim=False, uops_sha={})
    row = dve_ops._CUSTOM_DVE_ROW_BASE + len(dve_ops.OPS)
    assert row < 0x20
    dve_ops.OPS.append(op)
    dve_ops.CUSTOM_DVE_SPECS[name] = spec
    dve_ops._SUB_OPCODE_FOR_NAME[name] = row
    for ver in ("v3", "v4"):
        s = DveOpSpec(name=name, opcode=row, uops=lower(spec, ver=ver),
                      rd1_en=_has_src1(spec))
        object.__setattr__(op, "uops_sha", {**op.uops_sha, ver: s.sha(ver)})
    return op


def _build(S: int, R: int, C: int, n_act: int = 5, n_warm: int = 7,
           loop_iters: int | None = None):
    """S here = W (hot slots per group window)."""
    n_dve = S - n_act
    assert n_dve >= 0 and R == 128

    nc = bacc.Bacc("TRN2", target_bir_lowering=False, debug=False,
                   num_devices=N_CORES)

    FE = mybir.dt.float8e4
    U8 = mybir.dt.uint8
    BW = NGRP * S          # band-gram columns (4 groups x W)
    # packA: [lm2 | mb | band_rhs | rhsa] bytes on the sync queue
    PW = K * R + 2 * S * R // R * 1 * 2 + K * BW + K * NH  # fp8 sizes + mb f16
    PW = K * R * 1 + 2 * S + K * BW * 1 + K * NH * 1
    packA_d = nc.dram_tensor("packA", [128, PW], U8, kind="ExternalInput")
    rhsb_d = nc.dram_tensor("rhsb", [128, K * NH], FE, kind="ExternalInput")
    # rhsx: [rhs-extras(NB) | lhs-onehot(R) | lhs-nrm(R) | band-nrm(BW)]
    rhsx_d = nc.dram_tensor("rhsx", [33, NB + 2 * R + BW], F16,
                            kind="ExternalInput")
    out_d = nc.dram_tensor("out", [R, S + n_act], F32,
                           kind="ExternalOutput")

    with tile.TileContext(nc) as tc, ExitStack() as ctx:
        pool = ctx.enter_context(tc.tile_pool(name="sbuf", bufs=2))
        spool = ctx.enter_context(tc.tile_pool(name="scr", bufs=3))
        ppool = ctx.enter_context(tc.tile_pool(name="psum", bufs=2,
                                               space="PSUM"))

        def _body():
            rhsb_t = pool.tile([128, K, NH], FE)
            rhsx_t = pool.tile([33, NB + 2 * R + BW], F16)
            packA_t = pool.tile([128, PW], U8)
            nc.sync.dma_start(packA_t[:], packA_d.ap())
            o1 = K * R
            o2 = o1 + 2 * S
            o3 = o2 + K * BW
            lm2_t = packA_t[:, 0:o1].bitcast(FE)
            mb_t = packA_t[0:R, o1:o2].bitcast(F16)
            brhs_flat = packA_t[:, o2:o3].bitcast(FE)
            rhsa_flat = packA_t[:, o3:PW].bitcast(FE)
            nc.scalar.dma_start(rhsx_t[:], rhsx_d.ap())
            nc.scalar.dma_start(rhsb_t[:], rhsb_d.ap())

            lm2 = [lm2_t[:, k * R:(k + 1) * R] for k in range(K)]
            brhs = [brhs_flat[:, k * BW:(k + 1) * BW] for k in range(K)]
            rhs = {0: [rhsa_flat[:, k * NH:(k + 1) * NH] for k in range(K)],
                   1: [rhsb_t[:, k, :] for k in range(K)]}
            lhs5 = rhsx_t[0:33, NB:NB + R]
            lhs5_ones = rhsx_t[32:33, NB:NB + R]
            lhs6_nrm = rhsx_t[32:33, NB + R:NB + 2 * R]      # -2|a|^2
            bnrm = rhsx_t[32:33, NB + 2 * R:NB + 2 * R + BW]  # |p|^2
            rhs5 = {0: rhsx_t[0:33, 0:NH], 1: rhsx_t[0:33, NH:NB]}

            one16 = pool.tile([33, 1], F16)
            dumt = pool.tile([1, 2], F32)
            nc.vector.memset(one16[:], 1.0)
            nc.vector.memset(dumt[:], 1.0)
            dums = pool.tile([1, 2], F32)
            nc.scalar.activation(dums[:], dumt[:], Act.Sqrt)
            if n_warm:
                wsrc = pool.tile([128, NB], F16)
                nc.gpsimd.memset(wsrc[:], 0.0)
                warm = ppool.tile([1, NB], F32, tag="warm", name="warm",
                                  bufs=1)
                for w in range(n_warm):
                    nc.tensor.matmul(warm[:], wsrc[:, 0:1], wsrc[:],
                                     start=True, stop=True)

            # anchor-norm column from -2|a|^2 row
            nrma = ppool.tile([R, 1], F32, tag="nrma", name="nrma", bufs=1)
            nc.tensor.matmul(nrma[:], lhs6_nrm, one16[32:33, :],
                             start=True, stop=True)
            nrma_s = pool.tile([R, 1], F32)
            nc.vector.tensor_scalar(out=nrma_s[:], in0=nrma[:],
                                    scalar1=-0.5, scalar2=None, op0=Alu.mult)

            # ---- band gram: -2 a^T p + |p|^2  [R, 4*W] ----
            g_b = ppool.tile([R, BW], F32, tag="gb", name="gb")
            for k in range(K):
                nc.tensor.matmul(g_b[:], lm2[k], brhs[k],
                                 start=(k == 0), stop=False)
            nc.tensor.matmul(g_b[:], lhs5_ones, bnrm, start=False, stop=True)
            band2 = pool.tile([R, S], F32)
            for g in range(NGRP):
                nc.scalar.activation(
                    band2[g * 32:(g + 1) * 32, :],
                    g_b[g * 32:(g + 1) * 32, g * S:(g + 1) * S],
                    Act.Relu, bias=nrma_s[g * 32:(g + 1) * 32, :],
                    scale=1.0)
            dband = pool.tile([R, S], F32)
            nc.scalar.activation(dband[:], band2[:], Act.Sqrt)
            pos = pool.tile([R, S], F32)
            nc.vector.tensor_tensor(out=pos[:], in0=dband[:], in1=mb_t,
                                    op=Alu.add)

            # ---- n-side gram + dn (unchanged) ----
            g_n = ppool.tile([R, NB], F32, tag="gn", name="gn")
            for h in (0, 1):
                gh = g_n[:, h * NH:(h + 1) * NH]
                for k in range(K):
                    nc.tensor.matmul(gh, lm2[k], rhs[h][k],
                                     start=(k == 0), stop=False)
                nc.tensor.matmul(gh, lhs5, rhs5[h], start=False, stop=True)
            dn = pool.tile([R, NB], F16)
            nc.scalar.activation(dn[:], g_n[:], Act.Sqrt,
                                 bias=nrma_s[:], scale=1.0)

            # ---- hot loop ----
            fop = _register_fused_op()
            out_t = pool.tile([R, n_dve + 2 * n_act], F32)
            acc_a = pool.tile([R, 2 * max(n_act, 1)], F32)
            for j in range(S):
                pj = pos[:, j:j + 1]
                if j < n_dve:
                    z = spool.tile([R, NB], F16, tag="zd", name=f"zd_{j}")
                    nc.vector._custom_dve(
                        fop, out=z[:], in0=dn[:], s0=pj, s1=0.0,
                        imm2=CNT_SCALE, accum_out=out_t[:, j:j + 1])
                else:
                    ja = j - n_dve
                    z1 = ppool.tile([R, NB], F32, tag="za", bufs=2,
                                    name=f"za1_{j}")
                    nc.scalar.activation(z1[:], dn[:], Act.Relu,
                                         bias=pj, scale=-1.0,
                                         accum_out=acc_a[:, 2 * ja:2 * ja + 1])
                    z2 = ppool.tile([R, NB], F32, tag="za", bufs=2,
                                    name=f"za2_{j}")
                    nc.scalar.activation(z2[:], dn[:], Act.Sign,
                                         bias=pj, scale=-1.0,
                                         accum_out=acc_a[:, 2 * ja + 1:2 * ja + 2])
            if n_act:
                nc.vector.tensor_copy(
                    out_t[:, n_dve:n_dve + 2 * n_act], acc_a[:])
            nc.sync.dma_start(out_d.ap(), out_t[:])

        if loop_iters is None:
            _body()
        else:
            with tc.For_i(0, loop_iters, 1):
                _body()

    nc.compile()
    return nc


def _ilv(a, nchunk):
    x = a.shape[1]
    return np.ascontiguousarray(
        a.reshape(nchunk, 128, x).transpose(1, 0, 2).reshape(128, nchunk * x))


def _pack_bins(counts):
    """Pack class windows into 32 bins: rows(bin) = sum of member classes'
    counts <= 32; cols(bin) = sum of member chunk widths <= W. Big classes
    appear in multiple bins with disjoint column slices (all rows each).
    Randomized-restart greedy; deterministic (seeds tried in order)."""
    import random

    def attempt(seed):
        rng = random.Random(seed)
        rem = {cid: int(c) for cid, c in enumerate(counts)}
        cnt = dict(rem)
        bins = []
        for _ in range(N_CORES * NGRP):
            rows = cols = 0
            members = []
            while cols < W:
                placed = False
                cands = [(cid, c) for cid, c in rem.items() if c > 0]
                if not cands:
                    break
                rng.shuffle(cands)
                cands.sort(key=lambda x: (-x[1] + rng.random() * 2))
                for cid, c in cands:
                    if c <= W - cols and rows + cnt[cid] <= 32:
                        members.append((cid, cnt[cid] - rem[cid], c))
                        rows += cnt[cid]
                        cols += c
                        rem[cid] = 0
                        placed = True
                        break
                if placed:
                    continue
                for cid, c in cands:
                    take = min(c, W - cols)
                    if take > 0 and rows + cnt[cid] <= 32:
                        members.append((cid, cnt[cid] - rem[cid], take))
                        rows += cnt[cid]
                        cols += take
                        rem[cid] -= take
                        placed = True
                        break
                if not placed:
                    break
            bins.append(members)
        return bins, sum(rem.values())

    for seed in range(50000):
        bins, left = attempt(seed)
        if left == 0:
            return bins
    raise AssertionError("packing failed")


def _prepare(embeddings: np.ndarray, labels: np.ndarray):
    emb = np.ascontiguousarray(np.asarray(embeddings, dtype=np.float32))
    lab = np.asarray(labels)
    import ml_dtypes
    f8 = ml_dtypes.float8_e4m3fn

    perm = np.argsort(lab, kind="stable")
    e_s = emb[perm]
    lab_s = np.asarray(lab)[perm]
    classes, starts, counts = np.unique(lab_s, return_index=True,
                                        return_counts=True)
    C = len(classes)
    R = 128
    bins = _pack_bins(counts)

    cls_of_col = np.searchsorted(starts, np.arange(B), side="right") - 1
    e_sT = np.ascontiguousarray(e_s.T).astype(np.float32)   # [512, 512]
    sqn = (e_sT.astype(np.float16).astype(np.float32) ** 2).sum(0)

    # rhs extras (negatives): onehot + |b|^2 (same for all cores)
    ch_r = np.zeros((33, NB), dtype=np.float16)
    ch_r[cls_of_col[:NB], np.arange(NB)] = OH
    ch_r[32, :] = sqn.astype(np.float16)
    e8 = e_sT.astype(np.float16).astype(np.float32)
    rhsa = _ilv(e8[:, :NH].astype(f8), K)
    rhsb = _ilv(e8[:, NH:].astype(f8), K)

    num_valid = float((counts * (counts - 1) * (B - counts)).sum())
    BW = NGRP * W

    in_maps = []
    for b in range(N_CORES):
        core_bins = bins[NGRP * b:NGRP * b + NGRP]
        # anchors: 4 groups x 32 row-slots
        anchors = np.zeros(R, dtype=np.int64)      # col index into e_s
        live = np.zeros(R, dtype=bool)
        acls = np.full(R, -1, dtype=np.int64)
        bcols = np.zeros(BW, dtype=np.int64)
        bcls = np.full(BW, -1, dtype=np.int64)
        bliv = np.zeros(BW, dtype=bool)
        for g, members in enumerate(core_bins):
            r0 = 32 * g
            c0 = W * g
            ri = ci = 0
            for cid, coff, ncols in members:
                s0 = int(starts[cid])
                cn = int(counts[cid])
                anchors[r0 + ri:r0 + ri + cn] = np.arange(s0, s0 + cn)
                live[r0 + ri:r0 + ri + cn] = True
                acls[r0 + ri:r0 + ri + cn] = cid
                ri += cn
                bcols[c0 + ci:c0 + ci + ncols] = np.arange(
                    s0 + coff, s0 + coff + ncols)
                bcls[c0 + ci:c0 + ci + ncols] = cid
                bliv[c0 + ci:c0 + ci + ncols] = True
                ci += ncols
        eA = np.where(live[None, :], e8[:, anchors], 0.0)   # [512, 128]
        lm2 = _ilv((-2.0 * eA).astype(f8), K)               # [128, K*R]
        brhs = _ilv(np.where(bliv[None, :], e8[:, bcols], 0.0).astype(f8),
                    K)                                      # [128, K*BW]
        sqa = np.where(live, (eA ** 2).sum(0), 0.0)
        sqp = np.where(bliv, (e8[:, bcols] ** 2).sum(0), 0.0)

        ch = np.zeros((33, R), dtype=np.float16)            # lhs onehot
        ch[np.maximum(acls, 0), np.arange(R)] = np.where(live, OH, 0.0)
        ch[32, :] = np.float16(1.0)
        ch6 = np.zeros((33, R), dtype=np.float16)
        ch6[32, :] = (-2.0 * sqa).astype(np.float16)
        chb = np.zeros((33, BW), dtype=np.float16)          # band |p|^2
        chb[32, :] = sqp.astype(np.float16)
        rhsx = np.ascontiguousarray(
            np.concatenate([ch_r, ch, ch6, chb], axis=1))

        # mb: valid iff same class, anchor != col, both live
        gi = np.arange(R) // 32
        mb = np.full((R, W), NEG, dtype=np.float16)
        for r in range(R):
            if not live[r]:
                continue
            cw = slice(W * gi[r], W * gi[r] + W)
            ok = (bcls[cw] == acls[r]) & bliv[cw] & \
                 (bcols[cw] != anchors[r])
            mb[r, :] = np.where(ok, 0.0, NEG)

        mbu8 = np.ascontiguousarray(mb).view(np.uint8)      # [128, 2W]
        packA = np.concatenate(
            [np.ascontiguousarray(lm2).view(np.uint8), mbu8,
             np.ascontiguousarray(brhs).view(np.uint8),
             np.ascontiguousarray(rhsa).view(np.uint8)], axis=1)
        in_maps.append({
            "packA": np.ascontiguousarray(packA),
            "rhsb": rhsb,
            "rhsx": rhsx,
        })
    return W, R, C, in_maps, num_valid


def _combine(outs, num_valid, S, n_act=5):
    n_dve = S - n_act
    loss_sum = 0.0
    num_pos = 0.0
    R = outs[0].shape[0]
    for c in range(N_CORES):
        o = np.asarray(outs[c], dtype=np.float64)
        fused = o[:, 0:n_dve]
        fcnt = np.floor(fused / CNT_SCALE)
        loss_sum += (fused - CNT_SCALE * fcnt).sum()
        num_pos += fcnt.sum()
        for ja in range(n_act):
            c0 = n_dve + 2 * ja
            loss_sum += o[:, c0].sum()
            num_pos += 0.5 * (o[:, c0 + 1].sum() + NB * R)
    loss = np.float32(loss_sum / (num_pos + 1e-5))
    frac = np.float32(num_pos / (num_valid + 1e-5))
    return (loss, frac)


def kernel(embeddings: np.ndarray, labels: np.ndarray):
    S, R, C, in_maps, num_valid = _prepare(embeddings, labels)
    key = (S, R, C)
    if key not in _cache:
        _cache[key] = _build(S, R, C)
    nc = _cache[key]
    res = run_bass_kernel_spmd(nc, in_maps, core_ids=list(range(N_CORES)))
    return _combine([res.results[c]["out"] for c in range(N_CORES)],
                    num_valid, S)
